# revision 1
# baseline (speedup 1.0000x reference)
"""LoRA linear kernel for Trainium2, 8-core SPMD.

Computes out = x @ W^T + bias + (alpha/r) * (x @ A^T) @ B^T for
x [4, 4096, 4096], W [4096, 4096], A [16, 4096], B [4096, 16].

Sharding: data-parallel over tokens — each of the 8 cores owns 2048 of the
16384 flattened (B*S) tokens and the full output width.

Per-core kernel:
  - x shard cached entirely in SBUF as bf16 [128(kin), 32(ko), 2048(t)]
    (contraction dim on partitions), loaded once.
  - W streamed as bf16 [128, 16, 512] half-k column tiles, double-buffered
    (two half-k tiles = one full-k 512-wide column block).
  - out tile [128 t, 512 o]: 32 accumulating N=512 matmuls over k plus one
    extra matmul adding the LoRA term (rank 16 zero-padded to 128), all in
    one fp32 PSUM accumulation group.
  - xa^T = A @ x_shard^T computed on-device first ([16, 2048] fp32 -> bf16).
  - bias fused into the PSUM->SBUF eviction on VectorE; fp32 output.
"""

import sys

for _p in ("/opt/trn_rl_repo", "/root/.axon_site/_ro/trn_rl_repo"):
    if _p not in sys.path:
        sys.path.append(_p)

import numpy as np
from ml_dtypes import bfloat16

import concourse.bass as bass
import concourse.mybir as mybir
import concourse.tile as tile
from concourse import bacc
from concourse.bass import ts
from concourse.bass_utils import run_bass_kernel_spmd

# Problem shapes (hardcoded per contract)
B, S, D_IN, D_OUT = 4, 4096, 4096, 4096
R = 16
SCALE = 16.0 / 16.0
T = B * S                 # 16384 tokens
NCORES = 8
TC = T // NCORES          # 2048 tokens per core
P = 128
KO = D_IN // P            # 32 k-subtiles
KQ = 8                    # k-subtiles per W quarter tile
NQ = KO // KQ             # 4 quarter tiles per column block
NT = 512                  # output-column tile width
NNT = D_OUT // NT         # 8 n tiles
MS = TC // P              # 16 m subtiles per core

_BF = mybir.dt.bfloat16
_F32 = mybir.dt.float32


def _build_nc(repeat=1):
    nc = bacc.Bacc("TRN2", target_bir_lowering=False, debug=False,
                   num_devices=NCORES)

    xc_d = nc.dram_tensor("xc", [P, KO, TC], _BF, kind="ExternalInput").ap()
    w_d = nc.dram_tensor("wt", [NNT, NQ, P, KQ, NT], _BF,
                         kind="ExternalInput").ap()
    bias_d = nc.dram_tensor("biasr", [P, D_OUT], _F32, kind="ExternalInput").ap()
    at_d = nc.dram_tensor("at", [P, KO, R], _BF, kind="ExternalInput").ap()
    btp_d = nc.dram_tensor("btp", [P, D_OUT], _BF, kind="ExternalInput").ap()
    out_d = nc.dram_tensor("out", [TC, D_OUT], _F32, kind="ExternalOutput").ap()

    with tile.TileContext(nc) as tc:
        with (
            tc.tile_pool(name="xpool", bufs=1) as xpool,
            tc.tile_pool(name="wpool", bufs=5) as wpool,
            tc.tile_pool(name="cpool", bufs=1) as cpool,
            tc.tile_pool(name="bpool", bufs=2) as bpool,
            tc.tile_pool(name="opool", bufs=2) as opool,
            tc.tile_pool(name="psum", bufs=8, space="PSUM") as pp,
        ):
            for _ in range(repeat):
                # --- load x shard into SBUF as 16 independent chunk tiles
                # (2 k-subtiles each) so consumers only wait for the chunk
                # they read, not the whole 16.8 MB load.
                at_sb = cpool.tile([P, KO, R], _BF)
                nc.sync.dma_start(at_sb[:], at_d[:])
                KC = 2                    # k-subtiles per x chunk
                xcs = []
                for ck in range(KO // KC):
                    xt = xpool.tile([P, KC, TC], _BF, tag=f"x{ck}")
                    nc.sync.dma_start(xt[:], xc_d[:, ts(ck, KC)])
                    xcs.append(xt)

                def xck(ko):
                    return xcs[ko // KC][:, ko % KC]
                btp_sb = cpool.tile([P, D_OUT], _BF)
                nc.sync.dma_start(btp_sb[:], btp_d[:])

                # --- xa^T = A @ x_shard^T : [16, 2048], rank rows padded
                xaT_sb = cpool.tile([P, TC], _BF)
                nc.any.memzero(xaT_sb[:])
                for tt in range(TC // 512):
                    ps = pp.tile([P, 512], _F32, tag="ps")
                    for ko in range(KO):
                        nc.tensor.matmul(
                            ps[:R], at_sb[:, ko], xck(ko)[:, ts(tt, 512)],
                            start=(ko == 0), stop=(ko == KO - 1),
                        )
                    nc.vector.tensor_copy(xaT_sb[:R, ts(tt, 512)], ps[:R])

                # --- main loop: out[t, o] tiles
                for nt in range(NNT):
                    wq = []
                    for q in range(NQ):
                        w_sb = wpool.tile([P, KQ, NT], _BF, tag="w")
                        nc.sync.dma_start(w_sb[:], w_d[nt, q])
                        wq.append(w_sb)
                    bias_sb = bpool.tile([P, NT], _F32)
                    nc.sync.dma_start(bias_sb[:], bias_d[:, ts(nt, NT)])
                    for ms in range(MS):
                        ps = pp.tile([P, NT], _F32, tag="ps")
                        for ko in range(KO):
                            nc.tensor.matmul(
                                ps[:],
                                xck(ko)[:, ts(ms, P)],
                                wq[ko // KQ][:, ko % KQ],
                                start=(ko == 0), stop=False,
                            )
                        nc.tensor.matmul(
                            ps[:], xaT_sb[:, ts(ms, P)], btp_sb[:, ts(nt, NT)],
                            start=False, stop=True,
                        )
                        out_sb = opool.tile([P, NT], _F32)
                        nc.vector.tensor_add(out_sb[:], ps[:], bias_sb[:])
                        nc.sync.dma_start(out_d[ts(ms, P), ts(nt, NT)],
                                          out_sb[:])

    nc.compile()
    return nc


_NC_CACHE = None


def _get_nc():
    global _NC_CACHE
    if _NC_CACHE is None:
        _NC_CACHE = _build_nc()
    return _NC_CACHE


def _prep_inputs(x, weight, bias, lora_A, lora_B):
    xr = np.ascontiguousarray(x.reshape(T, D_IN))
    wt = np.ascontiguousarray(
        weight.reshape(NNT, NT, NQ, KQ, P).transpose(0, 2, 4, 3, 1)
    ).astype(bfloat16)
    biasr = np.ascontiguousarray(
        np.broadcast_to(bias.astype(np.float32), (P, D_OUT))
    )
    at = np.ascontiguousarray(
        lora_A.reshape(R, KO, P).transpose(2, 1, 0)
    ).astype(bfloat16)
    btp = np.zeros((P, D_OUT), dtype=bfloat16)
    btp[:R] = (SCALE * lora_B.T.astype(np.float32)).astype(bfloat16)

    in_maps = []
    for c in range(NCORES):
        xc = np.ascontiguousarray(
            xr[c * TC:(c + 1) * TC].reshape(TC, KO, P).transpose(2, 1, 0)
        ).astype(bfloat16)
        in_maps.append(
            {"xc": xc, "wt": wt, "biasr": biasr, "at": at, "btp": btp}
        )
    return in_maps


def run(inputs, trace=False):
    nc = _get_nc()
    in_maps = _prep_inputs(**inputs)
    res = run_bass_kernel_spmd(nc, in_maps, list(range(NCORES)), trace=trace)
    out = np.concatenate([r["out"] for r in res.results], axis=0)
    return out.reshape(B, S, D_OUT), res


def kernel(**inputs):
    out, _ = run(inputs, trace=False)
    return out



# revision 5
# speedup vs baseline: 1.9137x; 1.9137x over previous
"""LoRA linear kernel for Trainium2, 8-core SPMD.

Computes out = x @ W^T + bias + (alpha/r) * (x @ A^T) @ B^T for
x [4, 4096, 4096], W [4096, 4096], A [16, 4096], B [4096, 16].

Sharding: data-parallel over tokens — each of the 8 cores owns 2048 of the
16384 flattened (B*S) tokens and the full output width.

Per-core kernel (fp8 DoubleRow):
  - x shard stored in SBUF as TWO e4m3 tensors at scale 32: x_hi = q(32x)
    and x_lo = q(32x - x_hi), so x_hi + x_lo carries ~14 mantissa bits.
    Layout is token-block-major ([MS, P, KO, 128]) so compute on block ms
    only waits for that block's 1 MB of DMA, not the whole 16.8 MB shard.
  - W streamed as e4m3 at scale 2048. All main matmuls run in
    MatmulPerfMode.DoubleRow (256-deep contraction per instruction, 0.5
    cycles/row): per out tile [128t, 512o], 16 DR matmuls for x_hi plus 16
    for x_lo accumulate 65536*(x @ W^T) into fp32 PSUM. The quantization
    error is dominated by the single-e4m3 W (norm rel err ~1.55e-2 < 2e-2).
  - LoRA: xa^T = A @ x_shard^T computed per token block with the same
    hi/lo DR trick (A at scale 2048, 3 groups: hi*Ah, lo*Ah, hi*Al),
    evicted to bf16 at scale 65536 right before that block's first out
    tile. One extra bf16 matmul per out tile adds the rank-16 LoRA term
    AND the bias (bias rides row 16 of B^T against a constant 65536 row of
    xa^T, DMA-initialized) in the same PSUM group.
  - Eviction: single scalar-engine Copy with scale 2^-16; fp32 output.
"""

import sys

for _p in ("/opt/trn_rl_repo", "/root/.axon_site/_ro/trn_rl_repo"):
    if _p not in sys.path:
        sys.path.append(_p)

import numpy as np
from ml_dtypes import bfloat16, float8_e4m3

import concourse.bass as bass
import concourse.mybir as mybir
import concourse.tile as tile
from concourse import bacc
from concourse.bass import ts
from concourse.bass_utils import run_bass_kernel_spmd

# Problem shapes (hardcoded per contract)
B, S, D_IN, D_OUT = 4, 4096, 4096, 4096
R = 16
SCALE = 16.0 / 16.0
T = B * S                 # 16384 tokens
NCORES = 8
TC = T // NCORES          # 2048 tokens per core
P = 128
TB = 128                  # tokens per block
KO = D_IN // P            # 32 k-subtiles
KC = 2                    # k-subtiles per DoubleRow pair
NCH = KO // KC            # 16 pairs
KQ = 8                    # k-subtiles per W quarter tile
NQ = KO // KQ             # 4 quarter tiles per column block
NT = 512                  # output-column tile width
NNT = D_OUT // NT         # 8 n tiles
MS = TC // TB             # 16 m subtiles per core

XS = 32.0                 # x fp8 scale
WS = 2048.0               # W / A fp8 scale
PSC = XS * WS             # PSUM scale = 65536

_BF = mybir.dt.bfloat16
_F8 = mybir.dt.float8e4
_F32 = mybir.dt.float32
_DR = mybir.MatmulPerfMode.DoubleRow


def _build_nc(repeat=1):
    nc = bacc.Bacc("TRN2", target_bir_lowering=False, debug=False,
                   num_devices=NCORES)

    xh_d = nc.dram_tensor("xh", [MS, P, KO, TB], _F8, kind="ExternalInput").ap()
    xl_d = nc.dram_tensor("xl", [MS, P, KO, TB], _F8, kind="ExternalInput").ap()
    w_d = nc.dram_tensor("wt", [NNT, NQ, P, KQ, NT], _F8,
                         kind="ExternalInput").ap()
    ah_d = nc.dram_tensor("ah", [P, KO, R], _F8, kind="ExternalInput").ap()
    al_d = nc.dram_tensor("al", [P, KO, R], _F8, kind="ExternalInput").ap()
    btp_d = nc.dram_tensor("btp", [P, D_OUT], _BF, kind="ExternalInput").ap()
    out_d = nc.dram_tensor("out", [TC, D_OUT], _F32, kind="ExternalOutput").ap()

    with tile.TileContext(nc) as tc:
        with (
            tc.tile_pool(name="xpool", bufs=1) as xpool,
            tc.tile_pool(name="wpool", bufs=8) as wpool,
            tc.tile_pool(name="cpool", bufs=1) as cpool,
            tc.tile_pool(name="opool", bufs=3) as opool,
            tc.tile_pool(name="psum", bufs=6, space="PSUM") as pp,
            tc.tile_pool(name="psxa", bufs=2, space="PSUM") as pxa,
        ):
            for _ in range(repeat):
                ah_sb = cpool.tile([P, KO, R], _F8, tag="ah")
                nc.sync.dma_start(ah_sb[:], ah_d[:])
                al_sb = cpool.tile([P, KO, R], _F8, tag="al")
                nc.sync.dma_start(al_sb[:], al_d[:])
                btp_sb = cpool.tile([P, D_OUT], _BF, tag="btp")
                nc.sync.dma_start(btp_sb[:], btp_d[:])
                # xa^T at scale 65536: rows 0-15 written per block below;
                # row 16 = const 65536 (bias carrier), rest zero — all DMA'd.
                xaT_sb = cpool.tile([P, TC], _BF, tag="xaT")
                nc.sync.dma_start(xaT_sb[:], xinit_d[:])

                # First x block, then W for nt=0, then the bulk x stream —
                # so the first out tiles don't queue behind 16.8 MB of x.
                xh_t, xl_t = [], []
                for ms in range(1):
                    th = xpool.tile([P, KO, TB], _F8, tag=f"xh{ms}")
                    nc.sync.dma_start(th[:], xh_d[ms])
                    xh_t.append(th)
                    tl = xpool.tile([P, KO, TB], _F8, tag=f"xl{ms}")
                    nc.sync.dma_start(tl[:], xl_d[ms])
                    xl_t.append(tl)
                wq0 = []
                for q in range(NQ):
                    w_sb = wpool.tile([P, KQ, NT], _F8, tag="w")
                    nc.sync.dma_start(w_sb[:], w_d[0, q])
                    wq0.append(w_sb)
                for ms in range(1, MS):
                    th = xpool.tile([P, KO, TB], _F8, tag=f"xh{ms}")
                    nc.sync.dma_start(th[:], xh_d[ms])
                    xh_t.append(th)
                    tl = xpool.tile([P, KO, TB], _F8, tag=f"xl{ms}")
                    nc.sync.dma_start(tl[:], xl_d[ms])
                    xl_t.append(tl)

                for nt in range(NNT):
                    if nt == 0:
                        wq = wq0
                    else:
                        wq = []
                        for q in range(NQ):
                            w_sb = wpool.tile([P, KQ, NT], _F8, tag="w")
                            nc.sync.dma_start(w_sb[:], w_d[nt, q])
                            wq.append(w_sb)
                    for ms in range(MS):
                        if nt == 0:
                            # xa^T block: [16, 128] at scale 65536
                            pa = pxa.tile([P, TB], _F32, tag="ps")
                            for ck in range(NCH):
                                nc.tensor.matmul(
                                    pa[:R], ah_sb[:, ts(ck, KC)],
                                    xh_t[ms][:, ts(ck, KC)],
                                    start=(ck == 0), stop=False, perf_mode=_DR)
                                nc.tensor.matmul(
                                    pa[:R], ah_sb[:, ts(ck, KC)],
                                    xl_t[ms][:, ts(ck, KC)],
                                    start=False, stop=False, perf_mode=_DR)
                                nc.tensor.matmul(
                                    pa[:R], al_sb[:, ts(ck, KC)],
                                    xh_t[ms][:, ts(ck, KC)],
                                    start=False, stop=(ck == NCH - 1),
                                    perf_mode=_DR)
                            nc.vector.tensor_copy(xaT_sb[:R, ts(ms, TB)],
                                                  pa[:R])
                        ps = pp.tile([P, NT], _F32, tag="ps")
                        for ck in range(NCH):
                            q, kq = divmod(ck * KC, KQ)
                            nc.tensor.matmul(
                                ps[:], xh_t[ms][:, ts(ck, KC)],
                                wq[q][:, kq:kq + KC],
                                start=(ck == 0), stop=False, perf_mode=_DR)
                        for ck in range(NCH):
                            q, kq = divmod(ck * KC, KQ)
                            nc.tensor.matmul(
                                ps[:], xl_t[ms][:, ts(ck, KC)],
                                wq[q][:, kq:kq + KC],
                                start=False, stop=False, perf_mode=_DR)
                        # rank-16 LoRA + bias (row 16) in the same group
                        nc.tensor.matmul(
                            ps[:], xaT_sb[:, ts(ms, TB)],
                            btp_sb[:, ts(nt, NT)],
                            start=False, stop=True)
                        out_sb = opool.tile([P, NT], _F32)
                        nc.scalar.activation(
                            out_sb[:], ps[:],
                            mybir.ActivationFunctionType.Copy,
                            scale=1.0 / PSC)
                        nc.sync.dma_start(out_d[ts(ms, TB), ts(nt, NT)],
                                          out_sb[:])

    nc.compile()
    return nc


_NC_CACHE = None


def _get_nc():
    global _NC_CACHE
    if _NC_CACHE is None:
        _NC_CACHE = _build_nc()
    return _NC_CACHE


def _layout_x(xc8):
    """[TC, D_IN] -> [MS, P, KO, TB]"""
    return np.ascontiguousarray(
        xc8.reshape(MS, TB, KO, P).transpose(0, 3, 2, 1))


def _prep_inputs(x, weight, bias, lora_A, lora_B):
    f32 = np.float32
    xr = x.reshape(T, D_IN).astype(f32)
    wt = np.ascontiguousarray(
        (weight.astype(f32) * WS).astype(float8_e4m3)
        .reshape(NNT, NT, NQ, KQ, P).transpose(0, 2, 4, 3, 1))
    a_s = lora_A.astype(f32) * WS
    ah8 = a_s.astype(float8_e4m3)
    al8 = (a_s - ah8.astype(f32)).astype(float8_e4m3)
    ah = np.ascontiguousarray(ah8.reshape(R, KO, P).transpose(2, 1, 0))
    al = np.ascontiguousarray(al8.reshape(R, KO, P).transpose(2, 1, 0))
    btp = np.zeros((P, D_OUT), dtype=bfloat16)
    btp[:R] = (SCALE * lora_B.T.astype(f32)).astype(bfloat16)
    btp[R] = bias.astype(f32).astype(bfloat16)
    xinit = np.zeros((P, TC), dtype=bfloat16)
    xinit[R] = np.float32(PSC)

    in_maps = []
    for c in range(NCORES):
        xs = xr[c * TC:(c + 1) * TC] * np.float32(XS)
        xh8 = xs.astype(float8_e4m3)
        xl8 = (xs - xh8.astype(f32)).astype(float8_e4m3)
        in_maps.append({
            "xh": _layout_x(xh8), "xl": _layout_x(xl8),
            "wt": wt, "ah": ah, "al": al, "btp": btp, "xinit": xinit,
        })
    return in_maps


def run(inputs, trace=False):
    nc = _get_nc()
    in_maps = _prep_inputs(**inputs)
    res = run_bass_kernel_spmd(nc, in_maps, list(range(NCORES)), trace=trace)
    out = np.concatenate([r["out"] for r in res.results], axis=0)
    return out.reshape(B, S, D_OUT), res


def kernel(**inputs):
    out, _ = run(inputs, trace=False)
    return out


# revision 26
# speedup vs baseline: 2.0002x; 1.0452x over previous
"""LoRA linear kernel for Trainium2, 8-core SPMD.

Computes out = x @ W^T + bias + (alpha/r) * (x @ A^T) @ B^T for
x [4, 4096, 4096], W [4096, 4096], A [16, 4096], B [4096, 16].

Sharding: data-parallel over tokens — each of the 8 cores owns 2048 of the
16384 flattened (B*S) tokens and the full output width.

Per-core kernel (fp8 DoubleRow):
  - x shard stored in SBUF as TWO e4m3 tensors at scale 32: x_hi = q(32x)
    and x_lo = q(32x - x_hi), so x_hi + x_lo carries ~14 mantissa bits.
    Layout is token-block-major ([MS, P, KO, 128]) so compute on block ms
    only waits for that block's 1 MB of DMA, not the whole 16.8 MB shard.
  - W streamed as e4m3 at scale 2048. All main matmuls run in
    MatmulPerfMode.DoubleRow (256-deep contraction per instruction, 0.5
    cycles/row): per out tile [128t, 512o], 16 DR matmuls for x_hi plus 16
    for x_lo accumulate 65536*(x @ W^T) into fp32 PSUM. The quantization
    error is dominated by the single-e4m3 W (norm rel err ~1.55e-2 < 2e-2).
  - LoRA: xa^T = A @ x_shard^T computed per token block with the same
    hi/lo DR trick (A at scale 2048, 3 groups: hi*Ah, lo*Ah, hi*Al),
    evicted to bf16 at scale 65536 right before that block's first out
    tile. One extra bf16 matmul per out tile adds the rank-16 LoRA term
    AND the bias (bias rides row 16 of B^T against a constant 65536 row of
    xa^T, DMA-initialized) in the same PSUM group.
  - Eviction: single scalar-engine Copy with scale 2^-16; fp32 output.
"""

import sys

for _p in ("/opt/trn_rl_repo", "/root/.axon_site/_ro/trn_rl_repo"):
    if _p not in sys.path:
        sys.path.append(_p)

import numpy as np
from ml_dtypes import bfloat16, float8_e4m3

import concourse.bass as bass
import concourse.mybir as mybir
import concourse.tile as tile
from concourse import bacc
from concourse.bass import ts
from concourse.bass_utils import run_bass_kernel_spmd

# Problem shapes (hardcoded per contract)
B, S, D_IN, D_OUT = 4, 4096, 4096, 4096
R = 16
SCALE = 16.0 / 16.0
T = B * S                 # 16384 tokens
NCORES = 8
TC = T // NCORES          # 2048 tokens per core
P = 128
TB = 128                  # tokens per block
KO = D_IN // P            # 32 k-subtiles
KC = 2                    # k-subtiles per DoubleRow pair
NCH = KO // KC            # 16 pairs
KQ = 8                    # k-subtiles per W quarter tile
NQ = KO // KQ             # 4 quarter tiles per column block
NT = 512                  # output-column tile width
NNT = D_OUT // NT         # 8 n tiles
MS = TC // TB             # 16 m subtiles per core

XS = 32.0                 # x fp8 scale
WS = 2048.0               # W / A fp8 scale
PSC = XS * WS             # PSUM scale = 65536

_BF = mybir.dt.bfloat16
_F8 = mybir.dt.float8e4
_F32 = mybir.dt.float32
_DR = mybir.MatmulPerfMode.DoubleRow


def _build_nc(repeat=1):
    nc = bacc.Bacc("TRN2", target_bir_lowering=False, debug=False,
                   num_devices=NCORES)

    xh_d = nc.dram_tensor("xh", [MS, P, KO, TB], _F8, kind="ExternalInput").ap()
    xl_d = nc.dram_tensor("xl", [MS, P, KO, TB], _F8, kind="ExternalInput").ap()
    w_d = nc.dram_tensor("wt", [NNT, NQ, P, KQ, NT], _F8,
                         kind="ExternalInput").ap()
    ah_d = nc.dram_tensor("ah", [P, KO, R], _F8, kind="ExternalInput").ap()
    al_d = nc.dram_tensor("al", [P, KO, R], _F8, kind="ExternalInput").ap()
    btp_d = nc.dram_tensor("btp", [P, D_OUT], _BF, kind="ExternalInput").ap()
    out_d = nc.dram_tensor("out", [TC, D_OUT], _F32, kind="ExternalOutput").ap()

    with tile.TileContext(nc) as tc:
        with (
            tc.tile_pool(name="xpool", bufs=1) as xpool,
            tc.tile_pool(name="wpool", bufs=12) as wpool,
            tc.tile_pool(name="cpool", bufs=1) as cpool,
            tc.tile_pool(name="opool", bufs=3) as opool,
            tc.tile_pool(name="psum", bufs=6, space="PSUM") as pp,
            tc.tile_pool(name="psxa", bufs=2, space="PSUM") as pxa,
        ):
            for _ in range(repeat):
                ah_sb = cpool.tile([P, KO, R], _F8, tag="ah")
                al_sb = cpool.tile([P, KO, R], _F8, tag="al")
                # xa^T at scale 65536: rows 0-15 written per block below;
                # rows 32-63 = const 65536 (row 32 carries the bias against
                # btp row 32; engine partition starts must be 32-aligned).
                xaT_sb = cpool.tile([P, TC], _BF, tag="xaT")
                nc.any.memzero(xaT_sb[:])
                nc.any.memset(xaT_sb[32:64], PSC)
                btp_sb = cpool.tile([P, D_OUT], _BF, tag="btp")
                # staging for the xa transpose; columns R..31 stay zero
                stg_sb = cpool.tile([P, 32], _BF, tag="stg")
                nc.any.memzero(stg_sb[:])

                # DMA order matters: one FIFO queue feeds the DMA engines, so
                # the x stream is staggered into the compute loop below —
                # out-tile DMAs must not queue behind the whole 16.8 MB of x
                # or PSUM/out-buffer backpressure stalls the PE.
                xh_t, xl_t = [], []

                def load_x(ms):
                    th = xpool.tile([P, KO, TB], _F8, tag=f"xh{ms}")
                    nc.sync.dma_start(th[:], xh_d[ms])
                    xh_t.append([th])
                    tl = xpool.tile([P, KO, TB], _F8, tag=f"xl{ms}")
                    nc.sync.dma_start(tl[:], xl_d[ms])
                    xl_t.append([tl])

                def xs(tlist, ck):
                    # block 0 is split into two half-k tiles for fast start
                    if len(tlist) == 2:
                        return tlist[ck // (NCH // 2)][:, ts(ck % (NCH // 2),
                                                             KC)]
                    return tlist[0][:, ts(ck, KC)]

                def load_half(d, ms, half, tag):
                    t = xpool.tile([P, KO // 2, TB], _F8, tag=tag)
                    nc.sync.dma_start(
                        t[:], d[ms][:, half * (KO // 2):(half + 1) * (KO // 2)])
                    return t

                def load_w0q(q):
                    w_sb = wpool.tile([P, KQ, NT], _F8, tag="w")
                    nc.sync.dma_start(w_sb[:], w_d[0, q])
                    return w_sb

                # Block-0 halves interleaved with W0 quarters so the first
                # matmuls start after ~0.8 MB of DMA, not ~3.4 MB.
                xh0 = [load_half(xh_d, 0, 0, "xh0a")]
                wq0 = [load_w0q(0)]
                xh0.append(load_half(xh_d, 0, 1, "xh0b"))
                xh_t.append(xh0)
                nc.sync.dma_start(ah_sb[:], ah_d[:])
                nc.sync.dma_start(al_sb[:], al_d[:])
                xl0 = [load_half(xl_d, 0, 0, "xl0a")]
                wq0.append(load_w0q(1))
                xl0.append(load_half(xl_d, 0, 1, "xl0b"))
                xl_t.append(xl0)
                wq0.append(load_w0q(2))
                wq0.append(load_w0q(3))
                nc.sync.dma_start(btp_sb[:, ts(0, NT)], btp_d[:, ts(0, NT)])
                load_x(1)
                load_x(2)

                def load_w(nt):
                    wq = []
                    for q in range(NQ):
                        w_sb = wpool.tile([P, KQ, NT], _F8, tag="w")
                        nc.sync.dma_start(w_sb[:], w_d[nt, q])
                        wq.append(w_sb)
                    nc.sync.dma_start(btp_sb[:, ts(nt, NT)],
                                      btp_d[:, ts(nt, NT)])
                    return wq

                def emit_tile(nt, ms, wq):
                        ps = pp.tile([P, NT], _F32, tag="ps")
                        for ck in range(NCH):
                            q, kq = divmod(ck * KC, KQ)
                            nc.tensor.matmul(
                                ps[:], xs(xh_t[ms], ck),
                                wq[q][:, kq:kq + KC],
                                start=(ck == 0), stop=False, perf_mode=_DR)
                        if nt == 0:
                            # xa block [128t, 16r] at scale 65536, x as the
                            # stationary operand so the moving free dim is
                            # only R (8-cycle DR matmuls), then two DVE
                            # 32x32 block transposes into xa^T rows 0-31.
                            pa = pxa.tile([P, R], _F32, tag="ps")
                            for ck in range(NCH):
                                nc.tensor.matmul(
                                    pa[:], xs(xh_t[ms], ck),
                                    ah_sb[:, ts(ck, KC)],
                                    start=(ck == 0), stop=False, perf_mode=_DR)
                            for ck in range(NCH):
                                nc.tensor.matmul(
                                    pa[:], xs(xh_t[ms], ck),
                                    al_sb[:, ts(ck, KC)],
                                    start=False, stop=False, perf_mode=_DR)
                            for ck in range(NCH):
                                nc.tensor.matmul(
                                    pa[:], xs(xl_t[ms], ck),
                                    ah_sb[:, ts(ck, KC)],
                                    start=False, stop=(ck == NCH - 1),
                                    perf_mode=_DR)
                            nc.vector.tensor_copy(stg_sb[:, :R], pa[:])
                            for b in range(P // 32):
                                c0 = ms * TB + 32 * b
                                nc.vector.transpose(
                                    xaT_sb[0:32, c0:c0 + 32],
                                    stg_sb[32 * b:32 * (b + 1), :])
                        for ck in range(NCH):
                            q, kq = divmod(ck * KC, KQ)
                            nc.tensor.matmul(
                                ps[:], xs(xl_t[ms], ck),
                                wq[q][:, kq:kq + KC],
                                start=False, stop=False, perf_mode=_DR)
                        # rank-16 LoRA + bias (row 32) in the same group
                        nc.tensor.matmul(
                            ps[:], xaT_sb[:, ts(ms, TB)],
                            btp_sb[:, ts(nt, NT)],
                            start=False, stop=True)
                        last = nt == NNT - 1 and ms == MS - 1
                        for h in range(2 if last else 1):
                            hs = NT // 2 if last else NT
                            out_sb = opool.tile([P, hs], _F32,
                                                tag=f"o{hs}")
                            nc.scalar.activation(
                                out_sb[:], ps[:, h * hs:(h + 1) * hs],
                                mybir.ActivationFunctionType.Copy,
                                scale=1.0 / PSC)
                            nc.sync.dma_start(
                                out_d[ts(ms, TB),
                                      nt * NT + h * hs:nt * NT + (h + 1) * hs],
                                out_sb[:])

                # Phase 1: nt=0 ramps solo for SKEW blocks (W1 hasn't landed
                # yet — the first ~20us are DMA-bound), then nt=0/nt=1 tiles
                # interleave per block. x blocks and W(nt+1) are DMA'd from
                # inside the loop so per-tile out DMAs interleave into the
                # (FIFO) queue instead of queueing behind bulk inputs.
                SKEW = 5
                wq1 = wq_next = None
                for m in range(SKEW):
                    emit_tile(0, m, wq0)
                    if m + 3 < MS:
                        load_x(m + 3)
                    if m == 1:
                        wq1 = load_w(1)
                for ms in range(MS):
                    emit_tile(1, ms, wq1)
                    if ms + SKEW < MS:
                        emit_tile(0, ms + SKEW, wq0)
                        if ms + SKEW + 3 < MS:
                            load_x(ms + SKEW + 3)
                    if ms == 10:
                        wq_next = load_w(2)
                wq = wq_next
                for nt in range(2, NNT):
                    for ms in range(MS):
                        emit_tile(nt, ms, wq)
                        if ms == 8 and nt + 1 < NNT:
                            wq_next = load_w(nt + 1)
                    wq = wq_next

    nc.compile()
    return nc


_NC_CACHE = None


def _get_nc():
    global _NC_CACHE
    if _NC_CACHE is None:
        _NC_CACHE = _build_nc()
    return _NC_CACHE


def _layout_x(xc8):
    """[TC, D_IN] -> [MS, P, KO, TB]"""
    return np.ascontiguousarray(
        xc8.reshape(MS, TB, KO, P).transpose(0, 3, 2, 1))


def _prep_inputs(x, weight, bias, lora_A, lora_B):
    f32 = np.float32
    xr = x.reshape(T, D_IN).astype(f32)
    wt = np.ascontiguousarray(
        (weight.astype(f32) * WS).astype(float8_e4m3)
        .reshape(NNT, NT, NQ, KQ, P).transpose(0, 2, 4, 3, 1))
    a_s = lora_A.astype(f32) * WS
    ah8 = a_s.astype(float8_e4m3)
    al8 = (a_s - ah8.astype(f32)).astype(float8_e4m3)
    ah = np.ascontiguousarray(ah8.reshape(R, KO, P).transpose(2, 1, 0))
    al = np.ascontiguousarray(al8.reshape(R, KO, P).transpose(2, 1, 0))
    btp = np.zeros((P, D_OUT), dtype=bfloat16)
    btp[:R] = (SCALE * lora_B.T.astype(f32)).astype(bfloat16)
    btp[32] = bias.astype(f32).astype(bfloat16)

    in_maps = []
    for c in range(NCORES):
        xs = xr[c * TC:(c + 1) * TC] * np.float32(XS)
        xh8 = xs.astype(float8_e4m3)
        xl8 = (xs - xh8.astype(f32)).astype(float8_e4m3)
        in_maps.append({
            "xh": _layout_x(xh8), "xl": _layout_x(xl8),
            "wt": wt, "ah": ah, "al": al, "btp": btp,
        })
    return in_maps


def run(inputs, trace=False):
    nc = _get_nc()
    in_maps = _prep_inputs(**inputs)
    res = run_bass_kernel_spmd(nc, in_maps, list(range(NCORES)), trace=trace)
    out = np.concatenate([r["out"] for r in res.results], axis=0)
    return out.reshape(B, S, D_OUT), res


def kernel(**inputs):
    out, _ = run(inputs, trace=False)
    return out


# revision 28
# speedup vs baseline: 2.5401x; 1.2699x over previous
"""LoRA linear kernel for Trainium2, 8-core SPMD.

Computes out = x @ W^T + bias + (alpha/r) * (x @ A^T) @ B^T for
x [4, 4096, 4096], W [4096, 4096], A [16, 4096], B [4096, 16].

Sharding: data-parallel over tokens — each of the 8 cores owns 2048 of the
16384 flattened (B*S) tokens and the full output width.

Per-core kernel (fp8 DoubleRow):
  - x shard stored in SBUF as TWO e4m3 tensors at scale 32: x_hi = q(32x)
    and x_lo = q(32x - x_hi), so x_hi + x_lo carries ~14 mantissa bits.
    Layout is token-block-major ([MS, P, KO, 128]) so compute on block ms
    only waits for that block's 1 MB of DMA, not the whole 16.8 MB shard.
  - W streamed as e4m3 at scale 2048. All main matmuls run in
    MatmulPerfMode.DoubleRow (256-deep contraction per instruction, 0.5
    cycles/row): per out tile [128t, 512o], 16 DR matmuls for x_hi plus 16
    for x_lo accumulate 65536*(x @ W^T) into fp32 PSUM. The quantization
    error is dominated by the single-e4m3 W (norm rel err ~1.55e-2 < 2e-2).
  - LoRA: xa^T = A @ x_shard^T computed per token block with the same
    hi/lo DR trick (A at scale 2048, 3 groups: hi*Ah, lo*Ah, hi*Al),
    evicted to bf16 at scale 65536 right before that block's first out
    tile. One extra bf16 matmul per out tile adds the rank-16 LoRA term
    AND the bias (bias rides row 16 of B^T against a constant 65536 row of
    xa^T, DMA-initialized) in the same PSUM group.
  - Eviction: single scalar-engine Copy with scale 2^-16; fp32 output.
"""

import sys

for _p in ("/opt/trn_rl_repo", "/root/.axon_site/_ro/trn_rl_repo"):
    if _p not in sys.path:
        sys.path.append(_p)

import numpy as np
from ml_dtypes import bfloat16, float8_e4m3

import concourse.bass as bass
import concourse.mybir as mybir
import concourse.tile as tile
from concourse import bacc
from concourse.bass import ts
from concourse.bass_utils import run_bass_kernel_spmd

# Problem shapes (hardcoded per contract)
B, S, D_IN, D_OUT = 4, 4096, 4096, 4096
R = 16
SCALE = 16.0 / 16.0
T = B * S                 # 16384 tokens
NCORES = 8
TC = T // NCORES          # 2048 tokens per core
P = 128
TB = 128                  # tokens per block
KO = D_IN // P            # 32 k-subtiles
KC = 2                    # k-subtiles per DoubleRow pair
NCH = KO // KC            # 16 pairs
KQ = 8                    # k-subtiles per W quarter tile
NQ = KO // KQ             # 4 quarter tiles per column block
NT = 512                  # output-column tile width
NNT = D_OUT // NT         # 8 n tiles
MS = TC // TB             # 16 m subtiles per core

XS = 32.0                 # x fp8 scale
WS = 2048.0               # W / A fp8 scale
PSC = XS * WS             # PSUM scale = 65536
# x_lo correction coverage: quantization errors are iid across k, so
# covering LO_NCH of NCH k-pairs leaves x-side error sqrt(1-eta) of full.
# 8/16 measures 1.67e-2 on HW (vs 1.37e-2 at 16/16), still under 2e-2.
LO_NCH = 8

_BF = mybir.dt.bfloat16
_F8 = mybir.dt.float8e4
_F32 = mybir.dt.float32
_DR = mybir.MatmulPerfMode.DoubleRow


def _build_nc(repeat=1):
    nc = bacc.Bacc("TRN2", target_bir_lowering=False, debug=False,
                   num_devices=NCORES)

    xh_d = nc.dram_tensor("xh", [MS, P, KO, TB], _F8, kind="ExternalInput").ap()
    xl_d = nc.dram_tensor("xl", [MS, P, KO, TB], _F8, kind="ExternalInput").ap()
    w_d = nc.dram_tensor("wt", [NNT, NQ, P, KQ, NT], _F8,
                         kind="ExternalInput").ap()
    ah_d = nc.dram_tensor("ah", [P, KO, R], _F8, kind="ExternalInput").ap()
    al_d = nc.dram_tensor("al", [P, KO, R], _F8, kind="ExternalInput").ap()
    btp_d = nc.dram_tensor("btp", [P, D_OUT], _BF, kind="ExternalInput").ap()
    out_d = nc.dram_tensor("out", [TC, D_OUT], _F32, kind="ExternalOutput").ap()

    with tile.TileContext(nc) as tc:
        with (
            tc.tile_pool(name="xpool", bufs=1) as xpool,
            tc.tile_pool(name="wpool", bufs=12) as wpool,
            tc.tile_pool(name="cpool", bufs=1) as cpool,
            tc.tile_pool(name="opool", bufs=3) as opool,
            tc.tile_pool(name="psum", bufs=6, space="PSUM") as pp,
            tc.tile_pool(name="psxa", bufs=2, space="PSUM") as pxa,
        ):
            for _ in range(repeat):
                ah_sb = cpool.tile([P, KO, R], _F8, tag="ah")
                al_sb = cpool.tile([P, KO, R], _F8, tag="al")
                # xa^T at scale 65536: rows 0-15 written per block below;
                # rows 32-63 = const 65536 (row 32 carries the bias against
                # btp row 32; engine partition starts must be 32-aligned).
                xaT_sb = cpool.tile([P, TC], _BF, tag="xaT")
                nc.any.memzero(xaT_sb[:])
                nc.any.memset(xaT_sb[32:64], PSC)
                btp_sb = cpool.tile([P, D_OUT], _BF, tag="btp")
                # staging for the xa transpose; columns R..31 stay zero
                stg_sb = cpool.tile([P, 32], _BF, tag="stg")
                nc.any.memzero(stg_sb[:])

                # DMA order matters: one FIFO queue feeds the DMA engines, so
                # the x stream is staggered into the compute loop below —
                # out-tile DMAs must not queue behind the whole 16.8 MB of x
                # or PSUM/out-buffer backpressure stalls the PE.
                xh_t, xl_t = [], []

                def load_x(ms):
                    th = xpool.tile([P, KO, TB], _F8, tag=f"xh{ms}")
                    nc.sync.dma_start(th[:], xh_d[ms])
                    xh_t.append([th])
                    tl = xpool.tile([P, KO, TB], _F8, tag=f"xl{ms}")
                    nc.sync.dma_start(tl[:], xl_d[ms])
                    xl_t.append([tl])

                def xs(tlist, ck):
                    # block 0 is split into two half-k tiles for fast start
                    if len(tlist) == 2:
                        return tlist[ck // (NCH // 2)][:, ts(ck % (NCH // 2),
                                                             KC)]
                    return tlist[0][:, ts(ck, KC)]

                def load_half(d, ms, half, tag):
                    t = xpool.tile([P, KO // 2, TB], _F8, tag=tag)
                    nc.sync.dma_start(
                        t[:], d[ms][:, half * (KO // 2):(half + 1) * (KO // 2)])
                    return t

                def load_w0q(q):
                    w_sb = wpool.tile([P, KQ, NT], _F8, tag="w")
                    nc.sync.dma_start(w_sb[:], w_d[0, q])
                    return w_sb

                # Block-0 halves interleaved with W0 quarters so the first
                # matmuls start after ~0.8 MB of DMA, not ~3.4 MB.
                xh0 = [load_half(xh_d, 0, 0, "xh0a")]
                wq0 = [load_w0q(0)]
                xh0.append(load_half(xh_d, 0, 1, "xh0b"))
                xh_t.append(xh0)
                nc.sync.dma_start(ah_sb[:], ah_d[:])
                nc.sync.dma_start(al_sb[:], al_d[:])
                xl0 = [load_half(xl_d, 0, 0, "xl0a")]
                wq0.append(load_w0q(1))
                xl0.append(load_half(xl_d, 0, 1, "xl0b"))
                xl_t.append(xl0)
                wq0.append(load_w0q(2))
                wq0.append(load_w0q(3))
                nc.sync.dma_start(btp_sb[:, ts(0, NT)], btp_d[:, ts(0, NT)])
                load_x(1)
                load_x(2)

                def load_w(nt):
                    wq = []
                    for q in range(NQ):
                        w_sb = wpool.tile([P, KQ, NT], _F8, tag="w")
                        nc.sync.dma_start(w_sb[:], w_d[nt, q])
                        wq.append(w_sb)
                    nc.sync.dma_start(btp_sb[:, ts(nt, NT)],
                                      btp_d[:, ts(nt, NT)])
                    return wq

                def emit_tile(nt, ms, wq):
                        ps = pp.tile([P, NT], _F32, tag="ps")
                        for ck in range(NCH):
                            q, kq = divmod(ck * KC, KQ)
                            nc.tensor.matmul(
                                ps[:], xs(xh_t[ms], ck),
                                wq[q][:, kq:kq + KC],
                                start=(ck == 0), stop=False, perf_mode=_DR)
                        if nt == 0:
                            # xa block [128t, 16r] at scale 65536, x as the
                            # stationary operand so the moving free dim is
                            # only R (8-cycle DR matmuls), then two DVE
                            # 32x32 block transposes into xa^T rows 0-31.
                            pa = pxa.tile([P, R], _F32, tag="ps")
                            for ck in range(NCH):
                                nc.tensor.matmul(
                                    pa[:], xs(xh_t[ms], ck),
                                    ah_sb[:, ts(ck, KC)],
                                    start=(ck == 0), stop=False, perf_mode=_DR)
                            for ck in range(NCH):
                                nc.tensor.matmul(
                                    pa[:], xs(xh_t[ms], ck),
                                    al_sb[:, ts(ck, KC)],
                                    start=False, stop=False, perf_mode=_DR)
                            for ck in range(NCH):
                                nc.tensor.matmul(
                                    pa[:], xs(xl_t[ms], ck),
                                    ah_sb[:, ts(ck, KC)],
                                    start=False, stop=(ck == NCH - 1),
                                    perf_mode=_DR)
                            nc.vector.tensor_copy(stg_sb[:, :R], pa[:])
                            for b in range(P // 32):
                                c0 = ms * TB + 32 * b
                                nc.vector.transpose(
                                    xaT_sb[0:32, c0:c0 + 32],
                                    stg_sb[32 * b:32 * (b + 1), :])
                        for ck in range(LO_NCH):
                            q, kq = divmod(ck * KC, KQ)
                            nc.tensor.matmul(
                                ps[:], xs(xl_t[ms], ck),
                                wq[q][:, kq:kq + KC],
                                start=False, stop=False, perf_mode=_DR)
                        # rank-16 LoRA + bias (row 32) in the same group
                        nc.tensor.matmul(
                            ps[:], xaT_sb[:, ts(ms, TB)],
                            btp_sb[:, ts(nt, NT)],
                            start=False, stop=True)
                        last = nt == NNT - 1 and ms == MS - 1
                        for h in range(2 if last else 1):
                            hs = NT // 2 if last else NT
                            out_sb = opool.tile([P, hs], _F32,
                                                tag=f"o{hs}")
                            nc.scalar.activation(
                                out_sb[:], ps[:, h * hs:(h + 1) * hs],
                                mybir.ActivationFunctionType.Copy,
                                scale=1.0 / PSC)
                            nc.sync.dma_start(
                                out_d[ts(ms, TB),
                                      nt * NT + h * hs:nt * NT + (h + 1) * hs],
                                out_sb[:])

                # Phase 1: nt=0 ramps solo for SKEW blocks (W1 hasn't landed
                # yet — the first ~20us are DMA-bound), then nt=0/nt=1 tiles
                # interleave per block. x blocks and W(nt+1) are DMA'd from
                # inside the loop so per-tile out DMAs interleave into the
                # (FIFO) queue instead of queueing behind bulk inputs.
                SKEW = 5
                wq1 = wq_next = None
                for m in range(SKEW):
                    emit_tile(0, m, wq0)
                    if m + 3 < MS:
                        load_x(m + 3)
                    if m == 1:
                        wq1 = load_w(1)
                for ms in range(MS):
                    emit_tile(1, ms, wq1)
                    if ms + SKEW < MS:
                        emit_tile(0, ms + SKEW, wq0)
                        if ms + SKEW + 3 < MS:
                            load_x(ms + SKEW + 3)
                    if ms == 10:
                        wq_next = load_w(2)
                wq = wq_next
                for nt in range(2, NNT):
                    for ms in range(MS):
                        emit_tile(nt, ms, wq)
                        if ms == 8 and nt + 1 < NNT:
                            wq_next = load_w(nt + 1)
                    wq = wq_next

    nc.compile()
    return nc


_NC_CACHE = None


def _get_nc():
    global _NC_CACHE
    if _NC_CACHE is None:
        _NC_CACHE = _build_nc()
    return _NC_CACHE


def _layout_x(xc8):
    """[TC, D_IN] -> [MS, P, KO, TB]"""
    return np.ascontiguousarray(
        xc8.reshape(MS, TB, KO, P).transpose(0, 3, 2, 1))


def _prep_inputs(x, weight, bias, lora_A, lora_B):
    f32 = np.float32
    xr = x.reshape(T, D_IN).astype(f32)
    wt = np.ascontiguousarray(
        (weight.astype(f32) * WS).astype(float8_e4m3)
        .reshape(NNT, NT, NQ, KQ, P).transpose(0, 2, 4, 3, 1))
    a_s = lora_A.astype(f32) * WS
    ah8 = a_s.astype(float8_e4m3)
    al8 = (a_s - ah8.astype(f32)).astype(float8_e4m3)
    ah = np.ascontiguousarray(ah8.reshape(R, KO, P).transpose(2, 1, 0))
    al = np.ascontiguousarray(al8.reshape(R, KO, P).transpose(2, 1, 0))
    btp = np.zeros((P, D_OUT), dtype=bfloat16)
    btp[:R] = (SCALE * lora_B.T.astype(f32)).astype(bfloat16)
    btp[32] = bias.astype(f32).astype(bfloat16)

    in_maps = []
    for c in range(NCORES):
        xs = xr[c * TC:(c + 1) * TC] * np.float32(XS)
        xh8 = xs.astype(float8_e4m3)
        xl8 = (xs - xh8.astype(f32)).astype(float8_e4m3)
        in_maps.append({
            "xh": _layout_x(xh8), "xl": _layout_x(xl8),
            "wt": wt, "ah": ah, "al": al, "btp": btp,
        })
    return in_maps


def run(inputs, trace=False):
    nc = _get_nc()
    in_maps = _prep_inputs(**inputs)
    res = run_bass_kernel_spmd(nc, in_maps, list(range(NCORES)), trace=trace)
    out = np.concatenate([r["out"] for r in res.results], axis=0)
    return out.reshape(B, S, D_OUT), res


def kernel(**inputs):
    out, _ = run(inputs, trace=False)
    return out
